# revision 14
# baseline (speedup 1.0000x reference)
"""Trainium2 Bass kernel for the adaLN (DiT-style) dense transformer block.

Sharding: data-parallel over B — core b computes batch element b (B=8, 8 cores,
no collectives). Host-side prep is layout-only: weight transposes + bf16 casts.

Per-core dataflow (T=2048 tokens, C=512, H=8 heads, DH=64, MLP=2048):
  - LN stats token-major (bn_stats over free dim, batched rstd); the modulated
    LN output is transposed to feature-major via the DMA xbar (zero PE cost)
    and the per-feature scale/shift (W, B) ride one per-partition tensor_scalar
  - attention per HEAD-PAIR: the two DH=64 score matmuls run concurrently on
    the PE's upper/lower 64-row tiles (tile_position derives from the operands'
    base partitions); both heads' logits land in one [128, 1024] PSUM tile so
    a single wide instruction computes exp for the pair
  - softmax exp is split across TWO engines: ScalarE Exp for half the tk steps,
    and a one-instruction Schraudolph approximation on VectorE for the rest
    (es = bf16_bits(int16(A*s + B)) ~= exp(s/8), max rel err ~3.3%; the common
    mode cancels in the softmax normalization)
  - o matmuls use lhsT=[v|ones] so the denominator rides the same matmul;
    normalization is deferred: unnormalized o + denominators are evacuated,
    reciprocals are batched (reciprocal_approx_fast) and applied as one
    broadcast multiply per head — replaces 64 serial [1,512] reciprocals
  - proj/fc2 run "swapped" (lhsT=activations) so outputs land token-major and
    the residual adds need no transpose
"""

import numpy as np
import ml_dtypes

import concourse.bass as bass
import concourse.bacc as bacc
import concourse.hw_specs as _hw_specs

# Route Exp and Ln to the one table set that holds BOTH
# (natural_log_exp_and_others). The default first-match assignment puts Exp in
# exp_and_others and Ln in natural_log, so every rstd = exp(-ln(v)/2) pair
# costs two 1.3us ACT table reloads. Blank those two sets (positions kept so
# act_func_set_ids stay aligned with act_info.json) and both functions
# first-match the combined set -> zero reloads.
if not getattr(_hw_specs.get_activation_tables, "_excl_exp_sets", False):
    _orig_get_tables = _hw_specs.get_activation_tables

    def _patched_get_tables(arch):
        t = _orig_get_tables(arch)
        for nm in ("exp_and_others", "natural_log"):
            if nm in t:
                t[nm] = set()
        return t

    _patched_get_tables._excl_exp_sets = True
    _hw_specs.get_activation_tables = _patched_get_tables
    bacc.get_activation_tables = _patched_get_tables
import concourse.tile as tile
import concourse.mybir as mybir
from concourse.bass_utils import run_bass_kernel_spmd

F32 = mybir.dt.float32
BF16 = mybir.dt.bfloat16
I16 = mybir.dt.int16
AF = mybir.ActivationFunctionType
ALU = mybir.AluOpType

B, T, C = 8, 2048, 512
H, DH, MLP = 8, 64, 4 * 512
P = 128
NT = T // P          # 16 token tiles
KC = C // P          # 4 feature chunks
NW = T // 512        # 4 tq windows of 512
EPS = 1e-5
GELU_AF = AF.Gelu_apprx_tanh  # test.py sim swaps to Tanh (CoreSim lacks gelu)

# Schraudolph bf16 exp: bf16_bits(int16(SCH_A*u + SCH_B)) ~= exp(u).
# SCH_B tuned for round-to-nearest (the HW convert; max rel 3.25%).
SCH_A = (2.0 ** 7) / np.log(2.0)
SCH_B = 16250.395
# tk steps whose exp runs on VectorE (rest on ScalarE). ~half each.
import os as _os
DVE_TKS = (frozenset() if _os.environ.get("K_NO_DVE_EXP")
           else frozenset(range(1, NT, 2)))
PE_TRANSPOSE = bool(_os.environ.get("K_PE_TRANSPOSE"))


def build_program():
    nc = bacc.Bacc("TRN2", target_bir_lowering=False, debug=False)

    # ---- DRAM I/O ----
    x_d = nc.dram_tensor("x", [NT, P, C], F32, kind="ExternalInput").ap()
    c_col = nc.dram_tensor("c_col", [P, KC], F32, kind="ExternalInput").ap()
    ada_wt = nc.dram_tensor("ada_wt", [KC, P, 6 * C], BF16, kind="ExternalInput").ap()
    qkv_wt = nc.dram_tensor("qkv_wt", [KC, P, 3 * C], BF16, kind="ExternalInput").ap()
    proj_wt = nc.dram_tensor("proj_wt", [KC, P, C], BF16, kind="ExternalInput").ap()
    fc1_wt = nc.dram_tensor("fc1_wt", [KC, P, MLP], BF16, kind="ExternalInput").ap()
    fc2_wt = nc.dram_tensor("fc2_wt", [MLP // P, P, C], BF16, kind="ExternalInput").ap()
    qkv_b_qk = nc.dram_tensor("qkv_b_qk", [P, 8], F32, kind="ExternalInput").ap()
    fc1_b_c = nc.dram_tensor("fc1_b_c", [P, MLP // P], F32, kind="ExternalInput").ap()
    # feature-major column constants [P, KC] per name, packed host-side:
    #   per branch br: A=ln_w, D=ln_w*(1+ada_b_sc), A2=ln_b,
    #   E=ln_b*(1+ada_b_sc)+ada_b_sh  (dev mod chunks complete them on-chip)
    lncols_d = nc.dram_tensor("lncols", [P, 8 * KC], F32, kind="ExternalInput").ap()
    # token-major broadcast rows [1, C]
    rows_d = {}
    for nm in ("vb_row", "gb1", "pb1", "gb2", "pb2"):
        rows_d[nm] = nc.dram_tensor(nm, [1, C], F32, kind="ExternalInput").ap()
    out_d = nc.dram_tensor("out", [NT, P, C], F32, kind="ExternalOutput").ap()
    # DRAM bounce buffers (partition-broadcast / transpose reads need DRAM src)
    mod_scr = nc.dram_tensor("mod_scr", [6, C], F32).ap()
    rec_scr = nc.dram_tensor("rec_scr", [H * NW, 512], BF16).ap()

    from contextlib import ExitStack
    with tile.TileContext(nc) as tc, ExitStack() as ctx:
        consts = ctx.enter_context(tc.tile_pool(name="consts", bufs=1))
        wbig = ctx.enter_context(tc.tile_pool(name="wbig", bufs=8))
        wsmall = ctx.enter_context(tc.tile_pool(name="wsmall", bufs=20))
        bigT = ctx.enter_context(tc.tile_pool(name="bigT", bufs=8))
        qk_pool = ctx.enter_context(tc.tile_pool(name="qk", bufs=8))
        vpool = ctx.enter_context(tc.tile_pool(name="vp", bufs=NT))
        es_pool = ctx.enter_context(tc.tile_pool(name="es", bufs=3))
        rbc_pool = ctx.enter_context(tc.tile_pool(name="rbc", bufs=2))
        work = ctx.enter_context(tc.tile_pool(name="work", bufs=2))
        psg = ctx.enter_context(tc.tile_pool(name="psg", bufs=2, space="PSUM"))
        pso = ctx.enter_context(tc.tile_pool(name="pso", bufs=4, space="PSUM"))

        # ---- persistent SBUF loads (ada first: it gates the mod chain) ----
        sc_col = consts.tile([P, KC], F32, name="sc_col")
        nc.sync.dma_start(sc_col, c_col)
        ada_sb = []
        for k in range(KC):
            halves = []
            for hh in range(2):
                w = wbig.tile([P, 3 * C], BF16, tag="wbig", name=f"ada{k}{hh}")
                nc.sync.dma_start(w, ada_wt[k][:, hh * 1536:(hh + 1) * 1536])
                halves.append(w)
            ada_sb.append(halves)
        sx = []
        for i in range(NT):
            t = consts.tile([P, C], F32, name=f"x{i}")
            nc.scalar.dma_start(t, x_d[i])
            sx.append(t)
        eps_t = consts.tile([P, 1], F32, name="eps_t")
        nc.gpsimd.memset(eps_t, EPS)
        qkvb_sb = consts.tile([P, 8], F32, name="qkvb_sb")
        nc.sync.dma_start(qkvb_sb, qkv_b_qk)
        fc1b_sb = consts.tile([P, MLP // P], F32, name="fc1b_sb")
        nc.sync.dma_start(fc1b_sb, fc1_b_c)
        lncols = consts.tile([P, 8 * KC], F32, name="lncols")
        nc.sync.dma_start(lncols, lncols_d)
        if PE_TRANSPOSE:
            from concourse.masks import make_identity
            ident = consts.tile([P, P], BF16, name="ident")
            make_identity(nc, ident)

        def lnc(idx):  # column group idx in the packed lncols tile
            return lncols[:, idx * KC:(idx + 1) * KC]

        # ---- phase 0: silu(c), mod = silu(c) @ ada_w.T (+ada_b host-folded) ----
        es_c = work.tile([P, KC], F32, tag="esc")
        nc.scalar.activation(es_c, sc_col, AF.Exp, scale=-1.0)
        nc.vector.tensor_scalar_add(es_c, es_c, 1.0)
        nc.vector.reciprocal(es_c, es_c)
        silu_f = work.tile([P, KC], F32, tag="siluf")
        nc.vector.tensor_mul(silu_f, sc_col, es_c)
        silu_b = consts.tile([P, KC], BF16, name="silu_b")
        nc.vector.tensor_copy(silu_b, silu_f)

        def bcast(dst, src_row):
            src = bass.AP(tensor=src_row.tensor, offset=src_row.offset,
                          ap=[[0, dst.shape[0]]] + list(src_row.ap[1:]))
            nc.sync.dma_start(out=dst, in_=src)

        def ada_mm_row(j):
            """mod chunk j (pre-ada_b) -> mod_scr[j] as a [1, C] DRAM row.
            chunks: 0=sh_msa 1=sc_msa 2=g_msa 3=sh_mlp 4=sc_mlp 5=g_mlp"""
            ps = pso.tile([P, 512], F32, tag="oac", name=f"adaps{j}")
            for k in range(KC):
                hh, off = divmod(j * C, 1536)
                nc.tensor.matmul(ps[0:1, 0:C], silu_b[:, k:k + 1],
                                 ada_sb[k][hh][:, off:off + C],
                                 start=(k == 0), stop=(k == KC - 1))
            mrow = work.tile([1, C], F32, tag="mrow", bufs=2, name=f"mrow{j}")
            nc.vector.tensor_copy(mrow, ps[0:1, 0:C])
            nc.sync.dma_start(mod_scr[j:j + 1, :], mrow)
            return mod_scr[j:j + 1, :]

        def col_read(row):
            """[1, C] DRAM row -> [P, KC] feature-major columns."""
            dst = work.tile([P, KC], F32, tag="colr", bufs=4)
            src = bass.AP(tensor=row.tensor, offset=row.offset,
                          ap=[[1, P], [P, KC]])
            nc.sync.dma_start(out=dst, in_=src)
            return dst

        def tmp_bc(src_row, nm):
            t = work.tile([P, C], F32, tag="tmp", bufs=3, name=nm)
            bcast(t, src_row)
            return t

        # modulation, feature-major columns: W = A*sc + D, B = A2*sc + sh + E
        # token-major broadcast tiles: G = g_dev + gb, GPB = G*pb
        Wcol, Bcol, Gt, GPBt = {}, {}, {}, {}
        for br in (1, 2):
            base = (br - 1) * 3
            lb = (br - 1) * 4
            sc_c = col_read(ada_mm_row(base + 1))
            sh_c = col_read(ada_mm_row(base + 0))
            Wc = consts.tile([P, KC], F32, name=f"W{br}c")
            nc.vector.tensor_mul(Wc, sc_c, lnc(lb + 0))
            nc.vector.tensor_add(Wc, Wc, lnc(lb + 1))
            Bc = consts.tile([P, KC], F32, name=f"B{br}c")
            nc.vector.tensor_mul(Bc, sc_c, lnc(lb + 2))
            nc.vector.tensor_add(Bc, Bc, lnc(lb + 3))
            nc.vector.tensor_add(Bc, Bc, sh_c)
            Wcol[br], Bcol[br] = Wc, Bc
            g_bc = tmp_bc(ada_mm_row(base + 2), f"gbc{br}")
            gb_bc = tmp_bc(rows_d[f"gb{br}"], f"gbbc{br}")
            G = consts.tile([P, C], BF16, name=f"G{br}")
            nc.vector.tensor_add(G, g_bc, gb_bc)
            pb_bc = tmp_bc(rows_d[f"pb{br}"], f"pbbc{br}")
            GPB = consts.tile([P, C], BF16, name=f"GPB{br}")
            nc.vector.tensor_mul(GPB, G, pb_bc)
            Gt[br], GPBt[br] = G, GPB
        VB = consts.tile([P, C], BF16, name="VB")
        vb_bc = tmp_bc(rows_d["vb_row"], "vbbc")
        nc.vector.tensor_copy(VB, vb_bc)

        # remaining weights (wbig slots 9-16 evict ada after its matmuls)
        qkv_sb = []
        for k in range(KC):
            w = wbig.tile([P, 3 * C], BF16, tag="wbig", name=f"qkvw{k}")
            nc.scalar.dma_start(w, qkv_wt[k])
            qkv_sb.append(w)
        fc1_sb = []
        for k in range(KC):
            w = wbig.tile([P, MLP], BF16, tag="wbig", name=f"fc1w{k}")
            nc.scalar.dma_start(w, fc1_wt[k])
            fc1_sb.append(w)
        proj_sb = []
        for k in range(KC):
            w = wsmall.tile([P, C], BF16, tag="wsmall", name=f"projw{k}")
            nc.scalar.dma_start(w, proj_wt[k])
            proj_sb.append(w)
        fc2_sb = []
        for k in range(MLP // P):
            w = wsmall.tile([P, C], BF16, tag="wsmall", name=f"fc2w{k}")
            nc.scalar.dma_start(w, fc2_wt[k])
            fc2_sb.append(w)

        # ---- LN: token-major stats (batched rstd), DMA-xbar transpose to
        # feature-major, then one per-partition tensor_scalar for W,B ----
        def ln_phase(tag, Wc, Bc, hT):
            mv = consts.tile([P, 2 * NT], F32, name=f"mv{tag}")
            for i in range(NT):
                st = work.tile([P, 6], F32, tag="st", bufs=2, name=f"st{tag}{i}")
                nc.vector.bn_stats(st, sx[i])
                nc.vector.bn_aggr(mv[:, 2 * i:2 * i + 2], st)
            mv3 = mv.rearrange("p (i two) -> p two i", two=2)
            rstd = consts.tile([P, NT], F32, name=f"rstd{tag}")
            nc.scalar.activation(rstd, mv3[:, 1, :], AF.Ln, bias=eps_t)
            nc.scalar.activation(rstd, rstd, AF.Exp, scale=-0.5)
            mr = consts.tile([P, NT], F32, name=f"mr{tag}")
            nc.vector.tensor_mul(mr, mv3[:, 0, :], rstd)
            for i in range(NT):
                t1 = work.tile([P, C], BF16, tag="t1", bufs=3, name=f"t1{tag}{i}")
                nc.vector.tensor_scalar(t1, sx[i], rstd[:, i:i + 1],
                                        mr[:, i:i + 1],
                                        op0=ALU.mult, op1=ALU.subtract)
                if PE_TRANSPOSE:
                    tp = pso.tile([P, 512], BF16, tag="oac", name=f"tp{tag}{i}")
                    for j in range(KC):
                        nc.tensor.transpose(tp[:, j * P:(j + 1) * P],
                                            t1[:, j * P:(j + 1) * P], ident)
                    for j in range(KC):
                        nc.vector.tensor_copy(hT[j][:, i * P:(i + 1) * P],
                                              tp[:, j * P:(j + 1) * P])
                else:
                    for j in range(KC):
                        nc.sync.dma_start_transpose(hT[j][:, i * P:(i + 1) * P],
                                                    t1[:, j * P:(j + 1) * P])
            for j in range(KC):
                nc.vector.tensor_scalar(hT[j], hT[j], Wc[:, j:j + 1],
                                        Bc[:, j:j + 1],
                                        op0=ALU.mult, op1=ALU.add)

        h1T = [bigT.tile([P, T], BF16, tag="bigT", name=f"h1T{j}") for j in range(KC)]
        ln_phase("a", Wcol[1], Bcol[1], h1T)

        # ---- qkv ----
        # v: token-major [tok, c_v] scattered into [128, 8, 65] (| ones)
        vtok = [vpool.tile([P, H * 65], BF16, tag="vtok", name=f"vtok{i}")
                for i in range(NT)]

        def v_mms():
            for i in range(NT):
                ps = pso.tile([P, 512], F32, tag="oac", name=f"vps{i}")
                for k in range(KC):
                    nc.tensor.matmul(ps, h1T[k][:, i * P:(i + 1) * P],
                                     qkv_sb[k][:, 2 * C:3 * C],
                                     start=(k == 0), stop=(k == KC - 1))
                src = ps.rearrange("p (h d) -> p h d", h=H)
                dst3 = vtok[i].rearrange("p (h d) -> p h d", d=65)[:, :, 0:DH]
                vb3 = VB.rearrange("p (h d) -> p h d", h=H)
                nc.vector.tensor_add(dst3, src, vb3)
                ones_col = vtok[i].rearrange("p (h d) -> p h d", d=65)[:, :, DH:65]
                nc.gpsimd.memset(ones_col, 1.0)

        qkT = {}

        def qk_mms(m):
            qkT[m] = qk_pool.tile([P, T], BF16, tag="qk", name=f"qkT{m}")
            prs = [psg.tile([P, 1024], F32, tag="sg", name=f"qkps{m}_{pp}")
                   for pp in range(2)]
            for k in range(KC):
                for n in range(NW):
                    nc.tensor.matmul(prs[n // 2][:, (n % 2) * 512:(n % 2) * 512 + 512],
                                     qkv_sb[k][:, m * P:(m + 1) * P],
                                     h1T[k][:, n * 512:(n + 1) * 512],
                                     start=(k == 0), stop=(k == KC - 1))
            for pp in range(2):
                nc.vector.tensor_scalar_add(qkT[m][:, pp * 1024:(pp + 1) * 1024],
                                            prs[pp], qkvb_sb[:, m:m + 1])

        # GPB1 fold: x += G1*proj_b runs on GpSimd during attention
        def gpb_fold(GPB):
            for i in range(NT):
                nc.gpsimd.tensor_add(sx[i], sx[i], GPB)

        oT = []

        def attention_pair(p):
            """Scores+softmax+o for heads (2p, 2p+1). Row-tiled score matmuls
            (the two 64-contraction matmuls run concurrently on the PE's
            upper/lower tiles); one exp instruction covers both heads."""
            he, ho = 2 * p, 2 * p + 1
            oTp = bigT.tile([P, T], BF16, tag="bigT", name=f"oT{p}")
            oT.append(oTp)
            kh, qh = qkT[4 + p], qkT[p]
            for w in range(NW):
                oac_e = pso.tile([P, 512], F32, tag="oac", name=f"oace{p}_{w}")
                oac_o = pso.tile([P, 512], F32, tag="oac", name=f"oaco{p}_{w}")
                es_prev = None

                def o_mms(tk, es):
                    nc.tensor.matmul(oac_e[0:65, :], vtok[tk][:, he * 65:he * 65 + 65],
                                     es[:, 0:512], start=(tk == 0), stop=(tk == NT - 1))
                    nc.tensor.matmul(oac_o[0:65, :], vtok[tk][:, ho * 65:ho * 65 + 65],
                                     es[:, 512:1024], start=(tk == 0), stop=(tk == NT - 1))

                for tk in range(NT):
                    sg = psg.tile([P, 1024], F32, tag="sg", name=f"sg{p}_{w}_{tk}")
                    nc.tensor.matmul(sg[:, 0:512], kh[0:64, tk * P:(tk + 1) * P],
                                     qh[0:64, w * 512:(w + 1) * 512],
                                     start=True, stop=True)
                    nc.tensor.matmul(sg[:, 512:1024], kh[64:128, tk * P:(tk + 1) * P],
                                     qh[64:128, w * 512:(w + 1) * 512],
                                     start=True, stop=True)
                    # o-matmuls run one tk behind so the in-order PE queue
                    # never waits on the exp of the current tk
                    if es_prev is not None:
                        o_mms(tk - 1, es_prev)
                    es = es_pool.tile([P, 1024], BF16, tag="es", name=f"es{p}_{w}_{tk}")
                    if tk in DVE_TKS:
                        nc.vector.tensor_scalar(es.bitcast(I16), sg,
                                                SCH_A * 0.125, SCH_B,
                                                op0=ALU.mult, op1=ALU.add)
                    else:
                        nc.scalar.activation(es, sg, AF.Exp, scale=0.125)
                    es_prev = es
                o_mms(NT - 1, es_prev)
                # evacuate: unnormalized o (ScalarE); denominator reciprocal
                # straight from the PSUM row (VectorE, deferred normalization)
                nc.scalar.copy(oTp[0:64, w * 512:(w + 1) * 512], oac_e[0:64, :])
                nc.scalar.copy(oTp[64:128, w * 512:(w + 1) * 512], oac_o[0:64, :])
                for h, oac in ((he, oac_e), (ho, oac_o)):
                    # custom-DVE ops can't read PSUM on HW: stage den in SBUF
                    dn = work.tile([1, 512], F32, tag="dn", bufs=2,
                                   name=f"dn{h}_{w}")
                    nc.vector.tensor_copy(dn, oac[64:65, :])
                    rw = work.tile([1, 512], F32, tag="rw", bufs=2,
                                   name=f"rw{h}_{w}")
                    nc.vector.reciprocal_approx_fast(rw, dn)
                    rwb = work.tile([1, 512], BF16, tag="rwb", bufs=2,
                                    name=f"rwb{h}_{w}")
                    nc.vector.tensor_copy(rwb, rw)
                    nc.sync.dma_start(rec_scr[h * NW + w:h * NW + w + 1, :], rwb)
            # softmax normalization for the pair (broadcast multiply);
            # rbc rows 0:64 = head-even rec, 64:128 = head-odd rec so each
            # tensor_tensor sees equal base partitions (walrus requirement)
            for w in range(NW):
                rbc = rbc_pool.tile([P, 512], BF16, tag="rbc", name=f"rbc{p}_{w}")
                bcast(rbc[0:DH, :], rec_scr[he * NW + w:he * NW + w + 1, :])
                bcast(rbc[DH:P, :], rec_scr[ho * NW + w:ho * NW + w + 1, :])
                nc.vector.tensor_mul(oTp[0:DH, w * 512:(w + 1) * 512],
                                     oTp[0:DH, w * 512:(w + 1) * 512],
                                     rbc[0:DH, :])
                nc.vector.tensor_mul(oTp[DH:P, w * 512:(w + 1) * 512],
                                     oTp[DH:P, w * 512:(w + 1) * 512],
                                     rbc[DH:P, :])

        # interleave: pair-0 attention starts as soon as its q/k and v exist
        qk_mms(0)
        qk_mms(4)
        v_mms()
        gpb_fold(GPBt[1])
        attention_pair(0)
        for p in range(1, 4):
            qk_mms(p)
            qk_mms(4 + p)
            attention_pair(p)

        # ---- proj (swapped: token-major out) + residual 1 (in-place x) ----
        for i in range(NT):
            ps = pso.tile([P, 512], F32, tag="oac", name=f"prps{i}")
            for k in range(KC):
                nc.tensor.matmul(ps, oT[k][:, i * P:(i + 1) * P],
                                 proj_sb[k], start=(k == 0), stop=(k == KC - 1))
            attn_sb = work.tile([P, C], BF16, tag="attnsb", bufs=2,
                                name=f"attnsb{i}")
            nc.vector.tensor_copy(attn_sb, ps)
            ta = work.tile([P, C], F32, tag="tmp", bufs=3, name=f"res1_{i}")
            nc.gpsimd.tensor_mul(ta, attn_sb, Gt[1])
            nc.vector.tensor_add(sx[i], sx[i], ta)

        # ---- LN2 (h2T reuses h1T slots) ----
        h2T = [bigT.tile([P, T], BF16, tag="bigT", name=f"h2T{j}") for j in range(KC)]
        ln_phase("b", Wcol[2], Bcol[2], h2T)
        # GPB2 fold after LN2 has consumed x2
        gpb_fold(GPBt[2])

        # ---- MLP per t-chunk; fc2 swapped -> token-major; residual 2 ----
        for n in range(NW):
            fps = [psg.tile([P, 1024], F32, tag="sg", name=f"fps{n}_{sp}")
                   for sp in range(2)]

            def fc2_mms(m, g1t):
                for s in range(4):
                    nc.tensor.matmul(fps[s // 2][:, (s % 2) * 512:(s % 2) * 512 + 512],
                                     g1t[:, s * P:(s + 1) * P], fc2_sb[m],
                                     start=(m == 0), stop=(m == MLP // P - 1))

            g1_prev = None
            for m in range(MLP // P):
                ps = pso.tile([P, 512], F32, tag="oac", name=f"f1ps{n}_{m}")
                for k in range(KC):
                    nc.tensor.matmul(ps, fc1_sb[k][:, m * P:(m + 1) * P],
                                     h2T[k][:, n * 512:(n + 1) * 512],
                                     start=(k == 0), stop=(k == KC - 1))
                if g1_prev is not None:
                    fc2_mms(m - 1, g1_prev)
                g1 = work.tile([P, C], BF16, tag="g1", bufs=3, name=f"g1_{n}_{m}")
                nc.scalar.activation(g1, ps, GELU_AF, bias=fc1b_sb[:, m:m + 1])
                g1_prev = g1
            fc2_mms(MLP // P - 1, g1_prev)
            for s in range(4):
                i = n * 4 + s
                mlp_sb = work.tile([P, C], BF16, tag="attnsb", bufs=2,
                                   name=f"mlpsb{i}")
                nc.vector.tensor_copy(mlp_sb,
                                      fps[s // 2][:, (s % 2) * 512:(s % 2) * 512 + 512])
                tb = work.tile([P, C], F32, tag="tmp", bufs=3, name=f"res2_{i}")
                nc.gpsimd.tensor_mul(tb, mlp_sb, Gt[2])
                nc.vector.tensor_add(sx[i], sx[i], tb)
                nc.sync.dma_start(out_d[i], sx[i])

    nc.compile()
    return nc


def make_in_maps(inputs):
    bf = ml_dtypes.bfloat16
    f32 = np.float32
    x = np.asarray(inputs["x"], f32)
    c = np.asarray(inputs["c"], f32)
    qkv_w = np.asarray(inputs["qkv_w"], f32)
    qkv_b = np.asarray(inputs["qkv_b"], f32)
    proj_w = np.asarray(inputs["proj_w"], f32)
    proj_b = np.asarray(inputs["proj_b"], f32)
    ada_w = np.asarray(inputs["ada_w"], f32)
    ada_b = np.asarray(inputs["ada_b"], f32)
    fc1_w = np.asarray(inputs["fc1_w"], f32)
    fc1_b = np.asarray(inputs["fc1_b"], f32)
    fc2_w = np.asarray(inputs["fc2_w"], f32)
    fc2_b = np.asarray(inputs["fc2_b"], f32)
    ln = {k: np.asarray(inputs[k], f32) for k in
          ["ln1_w", "ln1_b", "ln2_w", "ln2_b"]}

    shared = {
        "ada_wt": np.ascontiguousarray(ada_w.T.reshape(KC, P, 6 * C)).astype(bf),
        "qkv_wt": np.ascontiguousarray(qkv_w.T.reshape(KC, P, 3 * C)).astype(bf),
        "proj_wt": np.ascontiguousarray(proj_w.T.reshape(KC, P, C)).astype(bf),
        "fc1_wt": np.ascontiguousarray(fc1_w.T.reshape(KC, P, MLP)).astype(bf),
        "fc2_wt": np.ascontiguousarray(fc2_w.T.reshape(MLP // P, P, C)).astype(bf),
        "qkv_b_qk": np.ascontiguousarray(qkv_b[:2 * C].reshape(8, P).T).astype(f32),
        "fc1_b_c": np.ascontiguousarray(fc1_b.reshape(MLP // P, P).T).astype(f32),
        "vb_row": qkv_b[2 * C:].reshape(1, C).astype(f32),
    }
    # host-folded constants (weights-only algebra; inputs never touched):
    #   W = ln_w*(1+mod_sc) where mod_sc = dev_sc + ada_b_sc
    #     = dev_sc*A + D with A = ln_w, D = ln_w*(1+ada_b_sc); similarly B, G.
    # column layout: vec[c] at [c % 128, c // 128]
    def col(v):
        return np.ascontiguousarray(v.reshape(KC, P).T).astype(f32)

    lncols = []
    for br, (lnw, lnb, pb) in {1: (ln["ln1_w"], ln["ln1_b"], proj_b),
                               2: (ln["ln2_w"], ln["ln2_b"], fc2_b)}.items():
        o = (br - 1) * 3 * C
        sh_ab = ada_b[o:o + C]
        sc_ab = ada_b[o + C:o + 2 * C]
        g_ab = ada_b[o + 2 * C:o + 3 * C]
        lncols += [col(lnw), col(lnw * (1 + sc_ab)), col(lnb),
                   col(lnb * (1 + sc_ab) + sh_ab)]
        shared[f"gb{br}"] = g_ab.reshape(1, C).astype(f32)
        shared[f"pb{br}"] = pb.reshape(1, C).astype(f32)
    shared["lncols"] = np.ascontiguousarray(np.concatenate(lncols, axis=1))
    maps = []
    for b in range(B):
        m = dict(shared)
        m["x"] = np.ascontiguousarray(x[b].reshape(NT, P, C))
        m["c_col"] = np.ascontiguousarray(c[b].reshape(KC, P).T)
        maps.append(m)
    return maps


_CACHED_NC = None


def run(inputs, trace=False):
    global _CACHED_NC
    if _CACHED_NC is None:
        _CACHED_NC = build_program()
    maps = make_in_maps(inputs)
    res = run_bass_kernel_spmd(_CACHED_NC, maps, core_ids=list(range(B)),
                               trace=trace)
    out = np.stack([res.results[b]["out"].reshape(T, C) for b in range(B)])
    return out.astype(np.float32), res


def kernel(**inputs) -> np.ndarray:
    out, _ = run(inputs, trace=False)
    return out


# revision 19
# speedup vs baseline: 1.3307x; 1.3307x over previous
"""Trainium2 Bass kernel for the adaLN (DiT-style) dense transformer block.

Sharding: data-parallel over B — core b computes batch element b (B=8, 8 cores,
no collectives). Host-side prep is layout-only: weight transposes + bf16 casts.

Per-core dataflow (T=2048 tokens, C=512, H=8 heads, DH=64, MLP=2048):
  - LN stats token-major (bn_stats over free dim, batched rstd); the modulated
    LN output is transposed to feature-major via the DMA xbar (zero PE cost)
    and the per-feature scale/shift (W, B) ride one per-partition tensor_scalar
  - attention per HEAD-PAIR: the two DH=64 score matmuls run concurrently on
    the PE's upper/lower 64-row tiles (tile_position derives from the operands'
    base partitions); both heads' logits land in one [128, 1024] PSUM tile so
    a single wide instruction computes exp for the pair
  - softmax exp is split across TWO engines: ScalarE Exp for half the tk steps,
    and a one-instruction Schraudolph approximation on VectorE for the rest
    (es = bf16_bits(int16(A*s + B)) ~= exp(s/8), max rel err ~3.3%; the common
    mode cancels in the softmax normalization)
  - o matmuls use lhsT=[v|ones] so the denominator rides the same matmul;
    normalization is deferred: unnormalized o + denominators are evacuated,
    reciprocals are batched (reciprocal_approx_fast) and applied as one
    broadcast multiply per head — replaces 64 serial [1,512] reciprocals
  - proj/fc2 run "swapped" (lhsT=activations) so outputs land token-major and
    the residual adds need no transpose
"""

import numpy as np
import ml_dtypes

import concourse.bass as bass
import concourse.bacc as bacc
import concourse.hw_specs as _hw_specs

# Route Exp and Ln to the one table set that holds BOTH
# (natural_log_exp_and_others). The default first-match assignment puts Exp in
# exp_and_others and Ln in natural_log, so every rstd = exp(-ln(v)/2) pair
# costs two 1.3us ACT table reloads. Blank those two sets (positions kept so
# act_func_set_ids stay aligned with act_info.json) and both functions
# first-match the combined set -> zero reloads.
if not getattr(_hw_specs.get_activation_tables, "_excl_exp_sets", False):
    _orig_get_tables = _hw_specs.get_activation_tables

    def _patched_get_tables(arch):
        t = _orig_get_tables(arch)
        for nm in ("exp_and_others", "natural_log"):
            if nm in t:
                t[nm] = set()
        return t

    _patched_get_tables._excl_exp_sets = True
    _hw_specs.get_activation_tables = _patched_get_tables
    bacc.get_activation_tables = _patched_get_tables
import concourse.tile as tile
import concourse.mybir as mybir
from concourse.bass_utils import run_bass_kernel_spmd

F32 = mybir.dt.float32
BF16 = mybir.dt.bfloat16
I16 = mybir.dt.int16
AF = mybir.ActivationFunctionType
ALU = mybir.AluOpType

B, T, C = 8, 2048, 512
H, DH, MLP = 8, 64, 4 * 512
P = 128
NT = T // P          # 16 token tiles
KC = C // P          # 4 feature chunks
NW = T // 512        # 4 tq windows of 512
EPS = 1e-5
GELU_AF = AF.Gelu_apprx_tanh  # test.py sim swaps to Tanh (CoreSim lacks gelu)

# Schraudolph bf16 exp: bf16_bits(int16(SCH_A*u + SCH_B)) ~= exp(u).
# SCH_B tuned for round-to-nearest (the HW convert; max rel 3.25%).
SCH_A = (2.0 ** 7) / np.log(2.0)
SCH_B = 16250.395
# tk steps whose exp runs on VectorE (rest on ScalarE). ~half each.
import os as _os
DVE_TKS = (frozenset() if _os.environ.get("K_NO_DVE_EXP")
           else frozenset(range(1, NT, 2)))
PE_TRANSPOSE = not _os.environ.get("K_DMA_TRANSPOSE")


def build_program():
    nc = bacc.Bacc("TRN2", target_bir_lowering=False, debug=False)

    # ---- DRAM I/O ----
    x_d = nc.dram_tensor("x", [NT, P, C], F32, kind="ExternalInput").ap()
    c_col = nc.dram_tensor("c_col", [P, KC], F32, kind="ExternalInput").ap()
    ada_wt = nc.dram_tensor("ada_wt", [KC, P, 6 * C], BF16, kind="ExternalInput").ap()
    qkv_wt = nc.dram_tensor("qkv_wt", [KC, P, 3 * C], BF16, kind="ExternalInput").ap()
    proj_wt = nc.dram_tensor("proj_wt", [KC, P, C], BF16, kind="ExternalInput").ap()
    fc1_wt = nc.dram_tensor("fc1_wt", [KC, P, MLP], BF16, kind="ExternalInput").ap()
    fc2_wt = nc.dram_tensor("fc2_wt", [MLP // P, P, C], BF16, kind="ExternalInput").ap()
    qkv_b_qk = nc.dram_tensor("qkv_b_qk", [P, 8], F32, kind="ExternalInput").ap()
    fc1_b_c = nc.dram_tensor("fc1_b_c", [P, MLP // P], F32, kind="ExternalInput").ap()
    # feature-major column constants [P, KC] per name, packed host-side:
    #   per branch br: A=ln_w, D=ln_w*(1+ada_b_sc), A2=ln_b,
    #   E=ln_b*(1+ada_b_sc)+ada_b_sh  (dev mod chunks complete them on-chip)
    lncols_d = nc.dram_tensor("lncols", [P, 8 * KC], F32, kind="ExternalInput").ap()
    # token-major broadcast rows [1, C]
    rows_d = {}
    for nm in ("vb_row", "gb1", "pb1", "gb2", "pb2"):
        rows_d[nm] = nc.dram_tensor(nm, [1, C], F32, kind="ExternalInput").ap()
    out_d = nc.dram_tensor("out", [NT, P, C], F32, kind="ExternalOutput").ap()
    # DRAM bounce buffers (partition-broadcast / transpose reads need DRAM src)
    mod_scr = nc.dram_tensor("mod_scr", [6, C], F32).ap()
    rec_scr = nc.dram_tensor("rec_scr", [H * NW, 512], BF16).ap()

    from contextlib import ExitStack
    with tile.TileContext(nc) as tc, ExitStack() as ctx:
        consts = ctx.enter_context(tc.tile_pool(name="consts", bufs=1))
        wbig = ctx.enter_context(tc.tile_pool(name="wbig", bufs=8))
        wsmall = ctx.enter_context(tc.tile_pool(name="wsmall", bufs=20))
        bigT = ctx.enter_context(tc.tile_pool(name="bigT", bufs=8))
        qk_pool = ctx.enter_context(tc.tile_pool(name="qk", bufs=8))
        vpool = ctx.enter_context(tc.tile_pool(name="vp", bufs=NT))
        es_pool = ctx.enter_context(tc.tile_pool(name="es", bufs=4))
        rbc_pool = ctx.enter_context(tc.tile_pool(name="rbc", bufs=2))
        work = ctx.enter_context(tc.tile_pool(name="work", bufs=2))
        psg = ctx.enter_context(tc.tile_pool(name="psg", bufs=2, space="PSUM"))
        pso = ctx.enter_context(tc.tile_pool(name="pso", bufs=4, space="PSUM"))

        # ---- persistent SBUF loads (ada first: it gates the mod chain) ----
        sc_col = consts.tile([P, KC], F32, name="sc_col")
        nc.sync.dma_start(sc_col, c_col)
        ada_sb = []
        for k in range(KC):
            halves = []
            for hh in range(2):
                w = wbig.tile([P, 3 * C], BF16, tag="wbig", name=f"ada{k}{hh}")
                nc.sync.dma_start(w, ada_wt[k][:, hh * 1536:(hh + 1) * 1536])
                halves.append(w)
            ada_sb.append(halves)
        sx = []
        for i in range(NT):
            t = consts.tile([P, C], F32, name=f"x{i}")
            nc.scalar.dma_start(t, x_d[i])
            sx.append(t)
        eps_t = consts.tile([P, 1], F32, name="eps_t")
        nc.gpsimd.memset(eps_t, EPS)
        qkvb_sb = consts.tile([P, 8], F32, name="qkvb_sb")
        nc.sync.dma_start(qkvb_sb, qkv_b_qk)
        fc1b_sb = consts.tile([P, MLP // P], F32, name="fc1b_sb")
        nc.sync.dma_start(fc1b_sb, fc1_b_c)
        lncols = consts.tile([P, 8 * KC], F32, name="lncols")
        nc.sync.dma_start(lncols, lncols_d)
        if PE_TRANSPOSE:
            from concourse.masks import make_identity
            ident = consts.tile([P, P], BF16, name="ident")
            make_identity(nc, ident)

        def lnc(idx):  # column group idx in the packed lncols tile
            return lncols[:, idx * KC:(idx + 1) * KC]

        # ---- phase 0: silu(c), mod = silu(c) @ ada_w.T (+ada_b host-folded) ----
        es_c = work.tile([P, KC], F32, tag="esc")
        nc.scalar.activation(es_c, sc_col, AF.Exp, scale=-1.0)
        nc.vector.tensor_scalar_add(es_c, es_c, 1.0)
        nc.vector.reciprocal(es_c, es_c)
        silu_f = work.tile([P, KC], F32, tag="siluf")
        nc.vector.tensor_mul(silu_f, sc_col, es_c)
        silu_b = consts.tile([P, KC], BF16, name="silu_b")
        nc.vector.tensor_copy(silu_b, silu_f)

        def bcast(dst, src_row):
            src = bass.AP(tensor=src_row.tensor, offset=src_row.offset,
                          ap=[[0, dst.shape[0]]] + list(src_row.ap[1:]))
            nc.sync.dma_start(out=dst, in_=src)

        def ada_mm_row(j):
            """mod chunk j (pre-ada_b) -> mod_scr[j] as a [1, C] DRAM row.
            chunks: 0=sh_msa 1=sc_msa 2=g_msa 3=sh_mlp 4=sc_mlp 5=g_mlp"""
            ps = pso.tile([P, 512], F32, tag="oac", name=f"adaps{j}")
            for k in range(KC):
                hh, off = divmod(j * C, 1536)
                nc.tensor.matmul(ps[0:1, 0:C], silu_b[:, k:k + 1],
                                 ada_sb[k][hh][:, off:off + C],
                                 start=(k == 0), stop=(k == KC - 1))
            mrow = work.tile([1, C], F32, tag="mrow", bufs=2, name=f"mrow{j}")
            nc.vector.tensor_copy(mrow, ps[0:1, 0:C])
            nc.sync.dma_start(mod_scr[j:j + 1, :], mrow)
            return mod_scr[j:j + 1, :]

        def col_read(row):
            """[1, C] DRAM row -> [P, KC] feature-major columns."""
            dst = work.tile([P, KC], F32, tag="colr", bufs=4)
            src = bass.AP(tensor=row.tensor, offset=row.offset,
                          ap=[[1, P], [P, KC]])
            nc.sync.dma_start(out=dst, in_=src)
            return dst

        def tmp_bc(src_row, nm):
            t = work.tile([P, C], F32, tag="tmp", bufs=3, name=nm)
            bcast(t, src_row)
            return t

        # modulation, feature-major columns: W = A*sc + D, B = A2*sc + sh + E
        # token-major broadcast tiles: G = g_dev + gb, GPB = G*pb
        Wcol, Bcol, Gt, GPBt = {}, {}, {}, {}
        for br in (1, 2):
            base = (br - 1) * 3
            lb = (br - 1) * 4
            sc_c = col_read(ada_mm_row(base + 1))
            sh_c = col_read(ada_mm_row(base + 0))
            Wc = consts.tile([P, KC], F32, name=f"W{br}c")
            nc.vector.tensor_mul(Wc, sc_c, lnc(lb + 0))
            nc.vector.tensor_add(Wc, Wc, lnc(lb + 1))
            Bc = consts.tile([P, KC], F32, name=f"B{br}c")
            nc.vector.tensor_mul(Bc, sc_c, lnc(lb + 2))
            nc.vector.tensor_add(Bc, Bc, lnc(lb + 3))
            nc.vector.tensor_add(Bc, Bc, sh_c)
            Wcol[br], Bcol[br] = Wc, Bc
            g_bc = tmp_bc(ada_mm_row(base + 2), f"gbc{br}")
            gb_bc = tmp_bc(rows_d[f"gb{br}"], f"gbbc{br}")
            G = consts.tile([P, C], BF16, name=f"G{br}")
            nc.vector.tensor_add(G, g_bc, gb_bc)
            pb_bc = tmp_bc(rows_d[f"pb{br}"], f"pbbc{br}")
            GPB = consts.tile([P, C], BF16, name=f"GPB{br}")
            nc.vector.tensor_mul(GPB, G, pb_bc)
            Gt[br], GPBt[br] = G, GPB
        VB = consts.tile([P, C], BF16, name="VB")
        vb_bc = tmp_bc(rows_d["vb_row"], "vbbc")
        nc.vector.tensor_copy(VB, vb_bc)

        # remaining weights (wbig slots 9-16 evict ada after its matmuls)
        qkv_sb = []
        for k in range(KC):
            w = wbig.tile([P, 3 * C], BF16, tag="wbig", name=f"qkvw{k}")
            nc.scalar.dma_start(w, qkv_wt[k])
            qkv_sb.append(w)
        fc1_sb = []
        for k in range(KC):
            w = wbig.tile([P, MLP], BF16, tag="wbig", name=f"fc1w{k}")
            nc.scalar.dma_start(w, fc1_wt[k])
            fc1_sb.append(w)
        proj_sb = []
        for k in range(KC):
            w = wsmall.tile([P, C], BF16, tag="wsmall", name=f"projw{k}")
            nc.scalar.dma_start(w, proj_wt[k])
            proj_sb.append(w)
        fc2_sb = []
        for k in range(MLP // P):
            w = wsmall.tile([P, C], BF16, tag="wsmall", name=f"fc2w{k}")
            nc.scalar.dma_start(w, fc2_wt[k])
            fc2_sb.append(w)

        # ---- LN: token-major stats (batched rstd), DMA-xbar transpose to
        # feature-major, then one per-partition tensor_scalar for W,B ----
        def ln_phase(tag, Wc, Bc, hT):
            # fully per-tile pipelined: stats(i) -> rstd(i) -> modulated
            # normalize(i) -> transpose(i); W,B applied feature-major after
            for i in range(NT):
                st = work.tile([P, 6], F32, tag="st", bufs=2, name=f"st{tag}{i}")
                nc.vector.bn_stats(st, sx[i])
                mv = work.tile([P, 2], F32, tag="mv", bufs=3, name=f"mv{tag}{i}")
                nc.vector.bn_aggr(mv, st)
                rstd = work.tile([P, 1], F32, tag="rstd", bufs=3,
                                 name=f"rstd{tag}{i}")
                nc.scalar.activation(rstd, mv[:, 1:2], AF.Ln, bias=eps_t)
                nc.scalar.activation(rstd, rstd, AF.Exp, scale=-0.5)
                mr = work.tile([P, 1], F32, tag="mr", bufs=3, name=f"mr{tag}{i}")
                nc.vector.tensor_mul(mr, mv[:, 0:1], rstd)
                t1 = work.tile([P, C], BF16, tag="t1", bufs=2, name=f"t1{tag}{i}")
                nc.vector.tensor_scalar(t1, sx[i], rstd, mr,
                                        op0=ALU.mult, op1=ALU.subtract)
                if PE_TRANSPOSE:
                    tp = pso.tile([P, 512], BF16, tag="oac", name=f"tp{tag}{i}")
                    for j in range(KC):
                        nc.tensor.transpose(tp[:, j * P:(j + 1) * P],
                                            t1[:, j * P:(j + 1) * P], ident)
                    for j in range(KC):
                        nc.vector.tensor_copy(hT[j][:, i * P:(i + 1) * P],
                                              tp[:, j * P:(j + 1) * P])
                else:
                    for j in range(KC):
                        nc.sync.dma_start_transpose(hT[j][:, i * P:(i + 1) * P],
                                                    t1[:, j * P:(j + 1) * P])
            for j in range(KC):
                nc.vector.tensor_scalar(hT[j], hT[j], Wc[:, j:j + 1],
                                        Bc[:, j:j + 1],
                                        op0=ALU.mult, op1=ALU.add)

        h1T = [bigT.tile([P, T], BF16, tag="bigT", name=f"h1T{j}") for j in range(KC)]
        ln_phase("a", Wcol[1], Bcol[1], h1T)

        # ---- qkv ----
        # v: token-major [tok, c_v] scattered into [128, 8, 65] (| ones)
        vtok = [vpool.tile([P, H * 65], BF16, tag="vtok", name=f"vtok{i}")
                for i in range(NT)]

        def v_mms():
            for i in range(NT):
                ps = pso.tile([P, 512], F32, tag="oac", name=f"vps{i}")
                for k in range(KC):
                    nc.tensor.matmul(ps, h1T[k][:, i * P:(i + 1) * P],
                                     qkv_sb[k][:, 2 * C:3 * C],
                                     start=(k == 0), stop=(k == KC - 1))
                src = ps.rearrange("p (h d) -> p h d", h=H)
                dst3 = vtok[i].rearrange("p (h d) -> p h d", d=65)[:, :, 0:DH]
                vb3 = VB.rearrange("p (h d) -> p h d", h=H)
                nc.vector.tensor_add(dst3, src, vb3)
                ones_col = vtok[i].rearrange("p (h d) -> p h d", d=65)[:, :, DH:65]
                nc.gpsimd.memset(ones_col, 1.0)

        qkT = {}

        def qk_mms(m):
            qkT[m] = qk_pool.tile([P, T], BF16, tag="qk", name=f"qkT{m}")
            prs = [psg.tile([P, 1024], F32, tag="sg", name=f"qkps{m}_{pp}")
                   for pp in range(2)]
            for k in range(KC):
                for n in range(NW):
                    nc.tensor.matmul(prs[n // 2][:, (n % 2) * 512:(n % 2) * 512 + 512],
                                     qkv_sb[k][:, m * P:(m + 1) * P],
                                     h1T[k][:, n * 512:(n + 1) * 512],
                                     start=(k == 0), stop=(k == KC - 1))
            for pp in range(2):
                nc.vector.tensor_scalar_add(qkT[m][:, pp * 1024:(pp + 1) * 1024],
                                            prs[pp], qkvb_sb[:, m:m + 1])

        # GPB1 fold: x += G1*proj_b runs on GpSimd during attention
        def gpb_fold(GPB):
            for i in range(NT):
                nc.gpsimd.tensor_add(sx[i], sx[i], GPB)

        oT = []

        def attention_pair(p):
            """Scores+softmax+o for heads (2p, 2p+1). Row-tiled score matmuls
            (the two 64-contraction matmuls run concurrently on the PE's
            upper/lower tiles); one exp instruction covers both heads."""
            he, ho = 2 * p, 2 * p + 1
            oTp = bigT.tile([P, T], BF16, tag="bigT", name=f"oT{p}")
            oT.append(oTp)
            kh, qh = qkT[4 + p], qkT[p]
            for w in range(NW):
                oac_e = pso.tile([P, 512], F32, tag="oac", name=f"oace{p}_{w}")
                oac_o = pso.tile([P, 512], F32, tag="oac", name=f"oaco{p}_{w}")
                es_hist = {}

                def o_mms(tk):
                    es = es_hist.pop(tk)
                    nc.tensor.matmul(oac_e[0:65, :], vtok[tk][:, he * 65:he * 65 + 65],
                                     es[:, 0:512], start=(tk == 0), stop=(tk == NT - 1))
                    nc.tensor.matmul(oac_o[0:65, :], vtok[tk][:, ho * 65:ho * 65 + 65],
                                     es[:, 512:1024], start=(tk == 0), stop=(tk == NT - 1))

                for tk in range(NT):
                    sg = psg.tile([P, 1024], F32, tag="sg", name=f"sg{p}_{w}_{tk}")
                    nc.tensor.matmul(sg[:, 0:512], kh[0:64, tk * P:(tk + 1) * P],
                                     qh[0:64, w * 512:(w + 1) * 512],
                                     start=True, stop=True)
                    nc.tensor.matmul(sg[:, 512:1024], kh[64:128, tk * P:(tk + 1) * P],
                                     qh[64:128, w * 512:(w + 1) * 512],
                                     start=True, stop=True)
                    # o-matmuls run TWO tk behind: exp(tk) on ScalarE and
                    # exp(tk+1) on VectorE overlap before o consumes them
                    if tk >= 2:
                        o_mms(tk - 2)
                    es = es_pool.tile([P, 1024], BF16, tag="es", name=f"es{p}_{w}_{tk}")
                    if tk in DVE_TKS:
                        nc.vector.tensor_scalar(es.bitcast(I16), sg,
                                                SCH_A * 0.125, SCH_B,
                                                op0=ALU.mult, op1=ALU.add)
                    else:
                        nc.scalar.activation(es, sg, AF.Exp, scale=0.125)
                    es_hist[tk] = es
                o_mms(NT - 2)
                o_mms(NT - 1)
                # evacuate: unnormalized o + den staging (ScalarE);
                # reciprocal on VectorE (custom op can't read PSUM on HW)
                nc.scalar.copy(oTp[0:64, w * 512:(w + 1) * 512], oac_e[0:64, :])
                nc.scalar.copy(oTp[64:128, w * 512:(w + 1) * 512], oac_o[0:64, :])
                for h, oac in ((he, oac_e), (ho, oac_o)):
                    dn = work.tile([1, 512], F32, tag="dn", bufs=2,
                                   name=f"dn{h}_{w}")
                    nc.scalar.copy(dn, oac[64:65, :])
                    rw = work.tile([1, 512], F32, tag="rw", bufs=2,
                                   name=f"rw{h}_{w}")
                    nc.vector.reciprocal_approx_fast(rw, dn)
                    rwb = work.tile([1, 512], BF16, tag="rwb", bufs=2,
                                    name=f"rwb{h}_{w}")
                    nc.scalar.copy(rwb, rw)
                    nc.sync.dma_start(rec_scr[h * NW + w:h * NW + w + 1, :], rwb)
            # softmax normalization for the pair (broadcast multiply);
            # rbc rows 0:64 = head-even rec, 64:128 = head-odd rec so each
            # tensor_tensor sees equal base partitions (walrus requirement)
            for w in range(NW):
                rbc = rbc_pool.tile([P, 512], BF16, tag="rbc", name=f"rbc{p}_{w}")
                bcast(rbc[0:DH, :], rec_scr[he * NW + w:he * NW + w + 1, :])
                bcast(rbc[DH:P, :], rec_scr[ho * NW + w:ho * NW + w + 1, :])
                nc.vector.tensor_mul(oTp[0:DH, w * 512:(w + 1) * 512],
                                     oTp[0:DH, w * 512:(w + 1) * 512],
                                     rbc[0:DH, :])
                nc.vector.tensor_mul(oTp[DH:P, w * 512:(w + 1) * 512],
                                     oTp[DH:P, w * 512:(w + 1) * 512],
                                     rbc[DH:P, :])

        # interleave: pair-0 attention starts as soon as its q/k and v exist
        qk_mms(0)
        qk_mms(4)
        v_mms()
        gpb_fold(GPBt[1])
        attention_pair(0)
        for p in range(1, 4):
            qk_mms(p)
            qk_mms(4 + p)
            attention_pair(p)

        # ---- proj (swapped: token-major out) + residual 1 (in-place x) ----
        for i in range(NT):
            ps = pso.tile([P, 512], F32, tag="oac", name=f"prps{i}")
            for k in range(KC):
                nc.tensor.matmul(ps, oT[k][:, i * P:(i + 1) * P],
                                 proj_sb[k], start=(k == 0), stop=(k == KC - 1))
            attn_sb = work.tile([P, C], BF16, tag="attnsb", bufs=2,
                                name=f"attnsb{i}")
            nc.vector.tensor_copy(attn_sb, ps)
            ta = work.tile([P, C], F32, tag="tmp", bufs=3, name=f"res1_{i}")
            nc.gpsimd.tensor_mul(ta, attn_sb, Gt[1])
            nc.vector.tensor_add(sx[i], sx[i], ta)

        # ---- LN2 (h2T reuses h1T slots) ----
        h2T = [bigT.tile([P, T], BF16, tag="bigT", name=f"h2T{j}") for j in range(KC)]
        ln_phase("b", Wcol[2], Bcol[2], h2T)
        # GPB2 fold after LN2 has consumed x2
        gpb_fold(GPBt[2])

        # ---- MLP per t-chunk; fc2 swapped -> token-major; residual 2 ----
        for n in range(NW):
            fps = [psg.tile([P, 1024], F32, tag="sg", name=f"fps{n}_{sp}")
                   for sp in range(2)]

            def fc2_mms(m, g1t):
                for s in range(4):
                    nc.tensor.matmul(fps[s // 2][:, (s % 2) * 512:(s % 2) * 512 + 512],
                                     g1t[:, s * P:(s + 1) * P], fc2_sb[m],
                                     start=(m == 0), stop=(m == MLP // P - 1))

            g1_prev = None
            for m in range(MLP // P):
                ps = pso.tile([P, 512], F32, tag="oac", name=f"f1ps{n}_{m}")
                for k in range(KC):
                    nc.tensor.matmul(ps, fc1_sb[k][:, m * P:(m + 1) * P],
                                     h2T[k][:, n * 512:(n + 1) * 512],
                                     start=(k == 0), stop=(k == KC - 1))
                if g1_prev is not None:
                    fc2_mms(m - 1, g1_prev)
                g1 = work.tile([P, C], BF16, tag="g1", bufs=3, name=f"g1_{n}_{m}")
                nc.scalar.activation(g1, ps, GELU_AF, bias=fc1b_sb[:, m:m + 1])
                g1_prev = g1
            fc2_mms(MLP // P - 1, g1_prev)
            for s in range(4):
                i = n * 4 + s
                mlp_sb = work.tile([P, C], BF16, tag="attnsb", bufs=2,
                                   name=f"mlpsb{i}")
                nc.vector.tensor_copy(mlp_sb,
                                      fps[s // 2][:, (s % 2) * 512:(s % 2) * 512 + 512])
                tb = work.tile([P, C], F32, tag="tmp", bufs=3, name=f"res2_{i}")
                nc.gpsimd.tensor_mul(tb, mlp_sb, Gt[2])
                nc.vector.tensor_add(sx[i], sx[i], tb)
                nc.sync.dma_start(out_d[i], sx[i])

    nc.compile()
    return nc


def make_in_maps(inputs):
    bf = ml_dtypes.bfloat16
    f32 = np.float32
    x = np.asarray(inputs["x"], f32)
    c = np.asarray(inputs["c"], f32)
    qkv_w = np.asarray(inputs["qkv_w"], f32)
    qkv_b = np.asarray(inputs["qkv_b"], f32)
    proj_w = np.asarray(inputs["proj_w"], f32)
    proj_b = np.asarray(inputs["proj_b"], f32)
    ada_w = np.asarray(inputs["ada_w"], f32)
    ada_b = np.asarray(inputs["ada_b"], f32)
    fc1_w = np.asarray(inputs["fc1_w"], f32)
    fc1_b = np.asarray(inputs["fc1_b"], f32)
    fc2_w = np.asarray(inputs["fc2_w"], f32)
    fc2_b = np.asarray(inputs["fc2_b"], f32)
    ln = {k: np.asarray(inputs[k], f32) for k in
          ["ln1_w", "ln1_b", "ln2_w", "ln2_b"]}

    shared = {
        "ada_wt": np.ascontiguousarray(ada_w.T.reshape(KC, P, 6 * C)).astype(bf),
        "qkv_wt": np.ascontiguousarray(qkv_w.T.reshape(KC, P, 3 * C)).astype(bf),
        "proj_wt": np.ascontiguousarray(proj_w.T.reshape(KC, P, C)).astype(bf),
        "fc1_wt": np.ascontiguousarray(fc1_w.T.reshape(KC, P, MLP)).astype(bf),
        "fc2_wt": np.ascontiguousarray(fc2_w.T.reshape(MLP // P, P, C)).astype(bf),
        "qkv_b_qk": np.ascontiguousarray(qkv_b[:2 * C].reshape(8, P).T).astype(f32),
        "fc1_b_c": np.ascontiguousarray(fc1_b.reshape(MLP // P, P).T).astype(f32),
        "vb_row": qkv_b[2 * C:].reshape(1, C).astype(f32),
    }
    # host-folded constants (weights-only algebra; inputs never touched):
    #   W = ln_w*(1+mod_sc) where mod_sc = dev_sc + ada_b_sc
    #     = dev_sc*A + D with A = ln_w, D = ln_w*(1+ada_b_sc); similarly B, G.
    # column layout: vec[c] at [c % 128, c // 128]
    def col(v):
        return np.ascontiguousarray(v.reshape(KC, P).T).astype(f32)

    lncols = []
    for br, (lnw, lnb, pb) in {1: (ln["ln1_w"], ln["ln1_b"], proj_b),
                               2: (ln["ln2_w"], ln["ln2_b"], fc2_b)}.items():
        o = (br - 1) * 3 * C
        sh_ab = ada_b[o:o + C]
        sc_ab = ada_b[o + C:o + 2 * C]
        g_ab = ada_b[o + 2 * C:o + 3 * C]
        lncols += [col(lnw), col(lnw * (1 + sc_ab)), col(lnb),
                   col(lnb * (1 + sc_ab) + sh_ab)]
        shared[f"gb{br}"] = g_ab.reshape(1, C).astype(f32)
        shared[f"pb{br}"] = pb.reshape(1, C).astype(f32)
    shared["lncols"] = np.ascontiguousarray(np.concatenate(lncols, axis=1))
    maps = []
    for b in range(B):
        m = dict(shared)
        m["x"] = np.ascontiguousarray(x[b].reshape(NT, P, C))
        m["c_col"] = np.ascontiguousarray(c[b].reshape(KC, P).T)
        maps.append(m)
    return maps


_CACHED_NC = None


def run(inputs, trace=False):
    global _CACHED_NC
    if _CACHED_NC is None:
        _CACHED_NC = build_program()
    maps = make_in_maps(inputs)
    res = run_bass_kernel_spmd(_CACHED_NC, maps, core_ids=list(range(B)),
                               trace=trace)
    out = np.stack([res.results[b]["out"].reshape(T, C) for b in range(B)])
    return out.astype(np.float32), res


def kernel(**inputs) -> np.ndarray:
    out, _ = run(inputs, trace=False)
    return out


# revision 25
# speedup vs baseline: 1.3630x; 1.0242x over previous
"""Trainium2 Bass kernel for the adaLN (DiT-style) dense transformer block.

Sharding: data-parallel over B — core b computes batch element b (B=8, 8 cores,
no collectives). Host-side prep is layout-only: weight transposes + bf16 casts.

Per-core dataflow (T=2048 tokens, C=512, H=8 heads, DH=64, MLP=2048):
  - LN stats token-major (bn_stats over free dim, batched rstd); the modulated
    LN output is transposed to feature-major via the DMA xbar (zero PE cost)
    and the per-feature scale/shift (W, B) ride one per-partition tensor_scalar
  - attention per HEAD-PAIR: the two DH=64 score matmuls run concurrently on
    the PE's upper/lower 64-row tiles (tile_position derives from the operands'
    base partitions); both heads' logits land in one [128, 1024] PSUM tile so
    a single wide instruction computes exp for the pair
  - softmax exp is split across TWO engines: ScalarE Exp for half the tk steps,
    and a one-instruction Schraudolph approximation on VectorE for the rest
    (es = bf16_bits(int16(A*s + B)) ~= exp(s/8), max rel err ~3.3%; the common
    mode cancels in the softmax normalization)
  - o matmuls use lhsT=[v|ones] so the denominator rides the same matmul;
    normalization is deferred: unnormalized o + denominators are evacuated,
    reciprocals are batched (reciprocal_approx_fast) and applied as one
    broadcast multiply per head — replaces 64 serial [1,512] reciprocals
  - proj/fc2 run "swapped" (lhsT=activations) so outputs land token-major and
    the residual adds need no transpose
"""

import numpy as np
import ml_dtypes

import concourse.bass as bass
import concourse.bacc as bacc
import concourse.hw_specs as _hw_specs

# Route Exp and Ln to the one table set that holds BOTH
# (natural_log_exp_and_others). The default first-match assignment puts Exp in
# exp_and_others and Ln in natural_log, so every rstd = exp(-ln(v)/2) pair
# costs two 1.3us ACT table reloads. Blank those two sets (positions kept so
# act_func_set_ids stay aligned with act_info.json) and both functions
# first-match the combined set -> zero reloads.
if not getattr(_hw_specs.get_activation_tables, "_excl_exp_sets", False):
    _orig_get_tables = _hw_specs.get_activation_tables

    def _patched_get_tables(arch):
        t = _orig_get_tables(arch)
        for nm in ("exp_and_others", "natural_log"):
            if nm in t:
                t[nm] = set()
        return t

    _patched_get_tables._excl_exp_sets = True
    _hw_specs.get_activation_tables = _patched_get_tables
    bacc.get_activation_tables = _patched_get_tables
import concourse.tile as tile
import concourse.mybir as mybir
from concourse.bass_utils import run_bass_kernel_spmd

F32 = mybir.dt.float32
BF16 = mybir.dt.bfloat16
I16 = mybir.dt.int16
AF = mybir.ActivationFunctionType
ALU = mybir.AluOpType

B, T, C = 8, 2048, 512
H, DH, MLP = 8, 64, 4 * 512
P = 128
NT = T // P          # 16 token tiles
KC = C // P          # 4 feature chunks
NW = T // 512        # 4 tq windows of 512
EPS = 1e-5
GELU_AF = AF.Gelu_apprx_tanh  # test.py sim swaps to Tanh (CoreSim lacks gelu)

# Schraudolph bf16 exp: bf16_bits(int16(SCH_A*u + SCH_B)) ~= exp(u).
# SCH_B tuned for round-to-nearest (the HW convert; max rel 3.25%).
SCH_A = (2.0 ** 7) / np.log(2.0)
SCH_B = 16250.395
# tk steps whose exp runs on VectorE (rest on ScalarE). ~half each.
import os as _os
DVE_TKS = (frozenset() if _os.environ.get("K_NO_DVE_EXP")
           else frozenset(range(1, NT, 2)))
PE_TRANSPOSE = not _os.environ.get("K_DMA_TRANSPOSE")


def build_program():
    nc = bacc.Bacc("TRN2", target_bir_lowering=False, debug=False)

    # ---- DRAM I/O ----
    x_d = nc.dram_tensor("x", [NT, P, C], F32, kind="ExternalInput").ap()
    c_col = nc.dram_tensor("c_col", [P, KC], F32, kind="ExternalInput").ap()
    ada_wt = nc.dram_tensor("ada_wt", [KC, P, 6 * C], BF16, kind="ExternalInput").ap()
    qkv_wt = nc.dram_tensor("qkv_wt", [KC, P, 3 * C], BF16, kind="ExternalInput").ap()
    proj_wt = nc.dram_tensor("proj_wt", [KC, P, C], BF16, kind="ExternalInput").ap()
    fc1_wt = nc.dram_tensor("fc1_wt", [KC, P, MLP], BF16, kind="ExternalInput").ap()
    fc2_wt = nc.dram_tensor("fc2_wt", [MLP // P, P, C], BF16, kind="ExternalInput").ap()
    qkv_b_qk = nc.dram_tensor("qkv_b_qk", [P, 8], F32, kind="ExternalInput").ap()
    fc1_b_c = nc.dram_tensor("fc1_b_c", [P, MLP // P], F32, kind="ExternalInput").ap()
    # feature-major column constants [P, KC] per name, packed host-side:
    #   per branch br: A=ln_w, D=ln_w*(1+ada_b_sc), A2=ln_b,
    #   E=ln_b*(1+ada_b_sc)+ada_b_sh  (dev mod chunks complete them on-chip)
    lncols_d = nc.dram_tensor("lncols", [P, 8 * KC], F32, kind="ExternalInput").ap()
    # token-major broadcast rows [1, C]
    rows_d = {}
    for nm in ("vb_row", "gb1", "pb1", "gb2", "pb2"):
        rows_d[nm] = nc.dram_tensor(nm, [1, C], F32, kind="ExternalInput").ap()
    out_d = nc.dram_tensor("out", [NT, P, C], F32, kind="ExternalOutput").ap()
    # DRAM bounce buffers (partition-broadcast / transpose reads need DRAM src)
    mod_scr = nc.dram_tensor("mod_scr", [6, C], F32).ap()
    rec_scr = nc.dram_tensor("rec_scr", [H * NW, 512], BF16).ap()

    from contextlib import ExitStack
    with tile.TileContext(nc) as tc, ExitStack() as ctx:
        consts = ctx.enter_context(tc.tile_pool(name="consts", bufs=1))
        wbig = ctx.enter_context(tc.tile_pool(name="wbig", bufs=8))
        wsmall = ctx.enter_context(tc.tile_pool(name="wsmall", bufs=20))
        bigT = ctx.enter_context(tc.tile_pool(name="bigT", bufs=8))
        qk_pool = ctx.enter_context(tc.tile_pool(name="qk", bufs=8))
        vpool = ctx.enter_context(tc.tile_pool(name="vp", bufs=NT))
        es_pool = ctx.enter_context(tc.tile_pool(name="es", bufs=4))
        rbc_pool = ctx.enter_context(tc.tile_pool(name="rbc", bufs=2))
        work = ctx.enter_context(tc.tile_pool(name="work", bufs=2))
        psg = ctx.enter_context(tc.tile_pool(name="psg", bufs=2, space="PSUM"))
        pso = ctx.enter_context(tc.tile_pool(name="pso", bufs=4, space="PSUM"))

        # ---- persistent SBUF loads (ada first: it gates the mod chain) ----
        sc_col = consts.tile([P, KC], F32, name="sc_col")
        nc.sync.dma_start(sc_col, c_col)
        ada_sb = []
        for k in range(KC):
            halves = []
            for hh in range(2):
                w = wbig.tile([P, 3 * C], BF16, tag="wbig", name=f"ada{k}{hh}")
                nc.sync.dma_start(w, ada_wt[k][:, hh * 1536:(hh + 1) * 1536])
                halves.append(w)
            ada_sb.append(halves)
        sx = []
        for i in range(NT):
            t = consts.tile([P, C], F32, name=f"x{i}")
            nc.scalar.dma_start(t, x_d[i])
            sx.append(t)
        eps_t = consts.tile([P, 1], F32, name="eps_t")
        nc.gpsimd.memset(eps_t, EPS)
        qkvb_sb = consts.tile([P, 8], F32, name="qkvb_sb")
        nc.sync.dma_start(qkvb_sb, qkv_b_qk)
        fc1b_sb = consts.tile([P, MLP // P], F32, name="fc1b_sb")
        nc.sync.dma_start(fc1b_sb, fc1_b_c)
        lncols = consts.tile([P, 8 * KC], F32, name="lncols")
        nc.sync.dma_start(lncols, lncols_d)
        if PE_TRANSPOSE:
            from concourse.masks import make_identity
            ident = consts.tile([P, P], BF16, name="ident")
            make_identity(nc, ident)

        def lnc(idx):  # column group idx in the packed lncols tile
            return lncols[:, idx * KC:(idx + 1) * KC]

        # ---- phase 0: silu(c), mod = silu(c) @ ada_w.T (+ada_b host-folded) ----
        es_c = work.tile([P, KC], F32, tag="esc")
        nc.scalar.activation(es_c, sc_col, AF.Exp, scale=-1.0)
        nc.vector.tensor_scalar_add(es_c, es_c, 1.0)
        nc.vector.reciprocal(es_c, es_c)
        silu_f = work.tile([P, KC], F32, tag="siluf")
        nc.vector.tensor_mul(silu_f, sc_col, es_c)
        silu_b = consts.tile([P, KC], BF16, name="silu_b")
        nc.vector.tensor_copy(silu_b, silu_f)

        def bcast(dst, src_row):
            src = bass.AP(tensor=src_row.tensor, offset=src_row.offset,
                          ap=[[0, dst.shape[0]]] + list(src_row.ap[1:]))
            nc.sync.dma_start(out=dst, in_=src)

        def ada_mm_row(j):
            """mod chunk j (pre-ada_b) -> mod_scr[j] as a [1, C] DRAM row.
            chunks: 0=sh_msa 1=sc_msa 2=g_msa 3=sh_mlp 4=sc_mlp 5=g_mlp"""
            ps = pso.tile([P, 512], F32, tag="oac", name=f"adaps{j}")
            for k in range(KC):
                hh, off = divmod(j * C, 1536)
                nc.tensor.matmul(ps[0:1, 0:C], silu_b[:, k:k + 1],
                                 ada_sb[k][hh][:, off:off + C],
                                 start=(k == 0), stop=(k == KC - 1))
            mrow = work.tile([1, C], F32, tag="mrow", bufs=2, name=f"mrow{j}")
            nc.vector.tensor_copy(mrow, ps[0:1, 0:C])
            nc.sync.dma_start(mod_scr[j:j + 1, :], mrow)
            return mod_scr[j:j + 1, :]

        def col_read(row):
            """[1, C] DRAM row -> [P, KC] feature-major columns."""
            dst = work.tile([P, KC], F32, tag="colr", bufs=4)
            src = bass.AP(tensor=row.tensor, offset=row.offset,
                          ap=[[1, P], [P, KC]])
            nc.sync.dma_start(out=dst, in_=src)
            return dst

        def tmp_bc(src_row, nm):
            t = work.tile([P, C], F32, tag="tmp", bufs=3, name=nm)
            bcast(t, src_row)
            return t

        # modulation, feature-major columns: W = A*sc + D, B = A2*sc + sh + E
        # token-major broadcast tiles: G = g_dev + gb, GPB = G*pb
        Wcol, Bcol, Gt, GPBt = {}, {}, {}, {}
        for br in (1, 2):
            base = (br - 1) * 3
            lb = (br - 1) * 4
            sc_c = col_read(ada_mm_row(base + 1))
            sh_c = col_read(ada_mm_row(base + 0))
            Wc = consts.tile([P, KC], F32, name=f"W{br}c")
            nc.vector.tensor_mul(Wc, sc_c, lnc(lb + 0))
            nc.vector.tensor_add(Wc, Wc, lnc(lb + 1))
            Bc = consts.tile([P, KC], F32, name=f"B{br}c")
            nc.vector.tensor_mul(Bc, sc_c, lnc(lb + 2))
            nc.vector.tensor_add(Bc, Bc, lnc(lb + 3))
            nc.vector.tensor_add(Bc, Bc, sh_c)
            Wcol[br], Bcol[br] = Wc, Bc
            g_bc = tmp_bc(ada_mm_row(base + 2), f"gbc{br}")
            gb_bc = tmp_bc(rows_d[f"gb{br}"], f"gbbc{br}")
            G = consts.tile([P, C], BF16, name=f"G{br}")
            nc.vector.tensor_add(G, g_bc, gb_bc)
            pb_bc = tmp_bc(rows_d[f"pb{br}"], f"pbbc{br}")
            GPB = consts.tile([P, C], BF16, name=f"GPB{br}")
            nc.vector.tensor_mul(GPB, G, pb_bc)
            Gt[br], GPBt[br] = G, GPB
        VB = consts.tile([P, C], BF16, name="VB")
        vb_bc = tmp_bc(rows_d["vb_row"], "vbbc")
        nc.vector.tensor_copy(VB, vb_bc)

        # remaining weights (wbig slots 9-16 evict ada after its matmuls)
        qkv_sb = []
        for k in range(KC):
            w = wbig.tile([P, 3 * C], BF16, tag="wbig", name=f"qkvw{k}")
            nc.scalar.dma_start(w, qkv_wt[k])
            qkv_sb.append(w)
        fc1_sb = []
        for k in range(KC):
            w = wbig.tile([P, MLP], BF16, tag="wbig", name=f"fc1w{k}")
            nc.scalar.dma_start(w, fc1_wt[k])
            fc1_sb.append(w)
        proj_sb = []
        for k in range(KC):
            w = wsmall.tile([P, C], BF16, tag="wsmall", name=f"projw{k}")
            nc.scalar.dma_start(w, proj_wt[k])
            proj_sb.append(w)
        fc2_sb = []
        for k in range(MLP // P):
            w = wsmall.tile([P, C], BF16, tag="wsmall", name=f"fc2w{k}")
            nc.scalar.dma_start(w, fc2_wt[k])
            fc2_sb.append(w)

        # ---- LN: token-major stats (batched rstd), DMA-xbar transpose to
        # feature-major, then one per-partition tensor_scalar for W,B ----
        def ln_phase(tag, Wc, Bc, hT):
            # fully per-tile pipelined: stats(i) -> rstd(i) -> modulated
            # normalize(i) -> transpose(i); W,B applied feature-major after
            for i in range(NT):
                st = work.tile([P, 6], F32, tag="st", bufs=2, name=f"st{tag}{i}")
                nc.vector.bn_stats(st, sx[i])
                mv = work.tile([P, 2], F32, tag="mv", bufs=3, name=f"mv{tag}{i}")
                nc.vector.bn_aggr(mv, st)
                rstd = work.tile([P, 1], F32, tag="rstd", bufs=3,
                                 name=f"rstd{tag}{i}")
                nc.scalar.activation(rstd, mv[:, 1:2], AF.Ln, bias=eps_t)
                nc.scalar.activation(rstd, rstd, AF.Exp, scale=-0.5)
                mr = work.tile([P, 1], F32, tag="mr", bufs=3, name=f"mr{tag}{i}")
                nc.vector.tensor_mul(mr, mv[:, 0:1], rstd)
                t1 = work.tile([P, C], BF16, tag="t1", bufs=2, name=f"t1{tag}{i}")
                nc.vector.tensor_scalar(t1, sx[i], rstd, mr,
                                        op0=ALU.mult, op1=ALU.subtract)
                if PE_TRANSPOSE:
                    tp = pso.tile([P, 512], BF16, tag="oac", name=f"tp{tag}{i}")
                    for j in range(KC):
                        nc.tensor.transpose(tp[:, j * P:(j + 1) * P],
                                            t1[:, j * P:(j + 1) * P], ident)
                    for j in range(KC):
                        nc.vector.tensor_copy(hT[j][:, i * P:(i + 1) * P],
                                              tp[:, j * P:(j + 1) * P])
                else:
                    for j in range(KC):
                        nc.sync.dma_start_transpose(hT[j][:, i * P:(i + 1) * P],
                                                    t1[:, j * P:(j + 1) * P])
            for j in range(KC):
                nc.vector.tensor_scalar(hT[j], hT[j], Wc[:, j:j + 1],
                                        Bc[:, j:j + 1],
                                        op0=ALU.mult, op1=ALU.add)

        h1T = [bigT.tile([P, T], BF16, tag="bigT", name=f"h1T{j}") for j in range(KC)]
        ln_phase("a", Wcol[1], Bcol[1], h1T)

        # ---- qkv ----
        # v: token-major [tok, c_v] scattered into [128, 8, 65] (| ones)
        vtok = [vpool.tile([P, H * 65], BF16, tag="vtok", name=f"vtok{i}")
                for i in range(NT)]

        def v_mms():
            for i in range(NT):
                ps = pso.tile([P, 512], F32, tag="oac", name=f"vps{i}")
                for k in range(KC):
                    nc.tensor.matmul(ps, h1T[k][:, i * P:(i + 1) * P],
                                     qkv_sb[k][:, 2 * C:3 * C],
                                     start=(k == 0), stop=(k == KC - 1))
                src = ps.rearrange("p (h d) -> p h d", h=H)
                dst3 = vtok[i].rearrange("p (h d) -> p h d", d=65)[:, :, 0:DH]
                vb3 = VB.rearrange("p (h d) -> p h d", h=H)
                nc.vector.tensor_add(dst3, src, vb3)
                ones_col = vtok[i].rearrange("p (h d) -> p h d", d=65)[:, :, DH:65]
                nc.gpsimd.memset(ones_col, 1.0)

        qkT = {}

        def qk_chunk(m, n):
            """qkT[m] columns n*512:(n+1)*512 — one window's worth, so these
            matmuls can interleave into the previous pair's attention."""
            if m not in qkT:
                qkT[m] = qk_pool.tile([P, T], BF16, tag="qk", name=f"qkT{m}")
            ps = pso.tile([P, 512], F32, tag="oac", name=f"qkps{m}_{n}")
            for k in range(KC):
                nc.tensor.matmul(ps, qkv_sb[k][:, m * P:(m + 1) * P],
                                 h1T[k][:, n * 512:(n + 1) * 512],
                                 start=(k == 0), stop=(k == KC - 1))
            nc.vector.tensor_scalar_add(qkT[m][:, n * 512:(n + 1) * 512],
                                        ps, qkvb_sb[:, m:m + 1])

        def qk_mms(m):
            for n in range(NW):
                qk_chunk(m, n)

        # GPB1 fold: x += G1*proj_b runs on GpSimd during attention
        def gpb_fold(GPB):
            for i in range(NT):
                nc.gpsimd.tensor_add(sx[i], sx[i], GPB)

        oT = []

        def attention_pair(p, next_qk=()):
            """Scores+softmax+o for heads (2p, 2p+1). Row-tiled score matmuls
            (the two 64-contraction matmuls run concurrently on the PE's
            upper/lower tiles); one exp instruction covers both heads."""
            he, ho = 2 * p, 2 * p + 1
            oTp = bigT.tile([P, T], BF16, tag="bigT", name=f"oT{p}")
            oT.append(oTp)
            kh, qh = qkT[4 + p], qkT[p]
            for w in range(NW):
                oac_e = pso.tile([P, 512], F32, tag="oac", name=f"oace{p}_{w}")
                oac_o = pso.tile([P, 512], F32, tag="oac", name=f"oaco{p}_{w}")
                es_hist = {}

                def o_mms(tk):
                    es = es_hist.pop(tk)
                    nc.tensor.matmul(oac_e[0:65, :], vtok[tk][:, he * 65:he * 65 + 65],
                                     es[:, 0:512], start=(tk == 0), stop=(tk == NT - 1))
                    nc.tensor.matmul(oac_o[0:65, :], vtok[tk][:, ho * 65:ho * 65 + 65],
                                     es[:, 512:1024], start=(tk == 0), stop=(tk == NT - 1))

                for tk in range(NT):
                    sg = psg.tile([P, 1024], F32, tag="sg", name=f"sg{p}_{w}_{tk}")
                    nc.tensor.matmul(sg[:, 0:512], kh[0:64, tk * P:(tk + 1) * P],
                                     qh[0:64, w * 512:(w + 1) * 512],
                                     start=True, stop=True)
                    nc.tensor.matmul(sg[:, 512:1024], kh[64:128, tk * P:(tk + 1) * P],
                                     qh[64:128, w * 512:(w + 1) * 512],
                                     start=True, stop=True)
                    # o-matmuls run TWO tk behind: exp(tk) on ScalarE and
                    # exp(tk+1) on VectorE overlap before o consumes them
                    if tk >= 2:
                        o_mms(tk - 2)
                    es = es_pool.tile([P, 1024], BF16, tag="es", name=f"es{p}_{w}_{tk}")
                    if tk in DVE_TKS:
                        nc.vector.tensor_scalar(es.bitcast(I16), sg,
                                                SCH_A * 0.125, SCH_B,
                                                op0=ALU.mult, op1=ALU.add)
                    else:
                        nc.scalar.activation(es, sg, AF.Exp, scale=0.125)
                    es_hist[tk] = es
                o_mms(NT - 2)
                o_mms(NT - 1)
                # evacuate: unnormalized o + den staging (ScalarE);
                # reciprocal on VectorE (custom op can't read PSUM on HW)
                nc.scalar.copy(oTp[0:64, w * 512:(w + 1) * 512], oac_e[0:64, :])
                nc.scalar.copy(oTp[64:128, w * 512:(w + 1) * 512], oac_o[0:64, :])
                for h, oac in ((he, oac_e), (ho, oac_o)):
                    dn = work.tile([1, 512], F32, tag="dn", bufs=2,
                                   name=f"dn{h}_{w}")
                    nc.scalar.copy(dn, oac[64:65, :])
                    rw = work.tile([1, 512], F32, tag="rw", bufs=2,
                                   name=f"rw{h}_{w}")
                    nc.vector.reciprocal_approx_fast(rw, dn)
                    rwb = work.tile([1, 512], BF16, tag="rwb", bufs=2,
                                    name=f"rwb{h}_{w}")
                    nc.scalar.copy(rwb, rw)
                    nc.sync.dma_start(rec_scr[h * NW + w:h * NW + w + 1, :], rwb)
                # next pair's q/k matmuls ride inside this pair's window so
                # the exp engines never go idle between pairs
                for m in next_qk:
                    qk_chunk(m, w)
            # softmax normalization for the pair (broadcast multiply);
            # rbc rows 0:64 = head-even rec, 64:128 = head-odd rec so each
            # tensor_tensor sees equal base partitions (walrus requirement)
            for w in range(NW):
                rbc = rbc_pool.tile([P, 512], BF16, tag="rbc", name=f"rbc{p}_{w}")
                bcast(rbc[0:DH, :], rec_scr[he * NW + w:he * NW + w + 1, :])
                bcast(rbc[DH:P, :], rec_scr[ho * NW + w:ho * NW + w + 1, :])
                nc.vector.tensor_mul(oTp[0:DH, w * 512:(w + 1) * 512],
                                     oTp[0:DH, w * 512:(w + 1) * 512],
                                     rbc[0:DH, :])
                nc.vector.tensor_mul(oTp[DH:P, w * 512:(w + 1) * 512],
                                     oTp[DH:P, w * 512:(w + 1) * 512],
                                     rbc[DH:P, :])

        # interleave: pair-0 attention starts as soon as its q/k and v exist;
        # later pairs' q/k matmuls stream inside the previous pair's windows
        qk_mms(0)
        qk_mms(4)
        v_mms()
        gpb_fold(GPBt[1])
        for p in range(4):
            attention_pair(p, next_qk=(p + 1, p + 5) if p < 3 else ())

        # ---- proj (swapped: token-major out) + residual 1 (in-place x) ----
        for i in range(NT):
            ps = pso.tile([P, 512], F32, tag="oac", name=f"prps{i}")
            for k in range(KC):
                nc.tensor.matmul(ps, oT[k][:, i * P:(i + 1) * P],
                                 proj_sb[k], start=(k == 0), stop=(k == KC - 1))
            attn_sb = work.tile([P, C], BF16, tag="attnsb", bufs=2,
                                name=f"attnsb{i}")
            nc.vector.tensor_copy(attn_sb, ps)
            ta = work.tile([P, C], F32, tag="tmp", bufs=3, name=f"res1_{i}")
            # alternate engines so the serial per-tile chain halves in wall time
            if i % 2 == 0:
                nc.gpsimd.tensor_mul(ta, attn_sb, Gt[1])
                nc.vector.tensor_add(sx[i], sx[i], ta)
            else:
                nc.vector.tensor_mul(ta, attn_sb, Gt[1])
                nc.gpsimd.tensor_add(sx[i], sx[i], ta)

        # ---- LN2 (h2T reuses h1T slots) ----
        h2T = [bigT.tile([P, T], BF16, tag="bigT", name=f"h2T{j}") for j in range(KC)]
        ln_phase("b", Wcol[2], Bcol[2], h2T)
        # GPB2 fold after LN2 has consumed x2
        gpb_fold(GPBt[2])

        # ---- MLP per t-chunk; fc2 swapped -> token-major; residual 2 ----
        for n in range(NW):
            fps = [psg.tile([P, 1024], F32, tag="sg", name=f"fps{n}_{sp}")
                   for sp in range(2)]

            def fc2_mms(m, g1t):
                for s in range(4):
                    nc.tensor.matmul(fps[s // 2][:, (s % 2) * 512:(s % 2) * 512 + 512],
                                     g1t[:, s * P:(s + 1) * P], fc2_sb[m],
                                     start=(m == 0), stop=(m == MLP // P - 1))

            g1_prev = None
            for m in range(MLP // P):
                ps = pso.tile([P, 512], F32, tag="oac", name=f"f1ps{n}_{m}")
                for k in range(KC):
                    nc.tensor.matmul(ps, fc1_sb[k][:, m * P:(m + 1) * P],
                                     h2T[k][:, n * 512:(n + 1) * 512],
                                     start=(k == 0), stop=(k == KC - 1))
                if g1_prev is not None:
                    fc2_mms(m - 1, g1_prev)
                g1 = work.tile([P, C], BF16, tag="g1", bufs=3, name=f"g1_{n}_{m}")
                nc.scalar.activation(g1, ps, GELU_AF, bias=fc1b_sb[:, m:m + 1])
                g1_prev = g1
            fc2_mms(MLP // P - 1, g1_prev)
            for s in range(4):
                i = n * 4 + s
                mlp_sb = work.tile([P, C], BF16, tag="attnsb", bufs=2,
                                   name=f"mlpsb{i}")
                nc.vector.tensor_copy(mlp_sb,
                                      fps[s // 2][:, (s % 2) * 512:(s % 2) * 512 + 512])
                tb = work.tile([P, C], F32, tag="tmp", bufs=3, name=f"res2_{i}")
                if i % 2 == 0:
                    nc.gpsimd.tensor_mul(tb, mlp_sb, Gt[2])
                    nc.vector.tensor_add(sx[i], sx[i], tb)
                else:
                    nc.vector.tensor_mul(tb, mlp_sb, Gt[2])
                    nc.gpsimd.tensor_add(sx[i], sx[i], tb)
                nc.sync.dma_start(out_d[i], sx[i])

    nc.compile()
    return nc


def make_in_maps(inputs):
    bf = ml_dtypes.bfloat16
    f32 = np.float32
    x = np.asarray(inputs["x"], f32)
    c = np.asarray(inputs["c"], f32)
    qkv_w = np.asarray(inputs["qkv_w"], f32)
    qkv_b = np.asarray(inputs["qkv_b"], f32)
    proj_w = np.asarray(inputs["proj_w"], f32)
    proj_b = np.asarray(inputs["proj_b"], f32)
    ada_w = np.asarray(inputs["ada_w"], f32)
    ada_b = np.asarray(inputs["ada_b"], f32)
    fc1_w = np.asarray(inputs["fc1_w"], f32)
    fc1_b = np.asarray(inputs["fc1_b"], f32)
    fc2_w = np.asarray(inputs["fc2_w"], f32)
    fc2_b = np.asarray(inputs["fc2_b"], f32)
    ln = {k: np.asarray(inputs[k], f32) for k in
          ["ln1_w", "ln1_b", "ln2_w", "ln2_b"]}

    shared = {
        "ada_wt": np.ascontiguousarray(ada_w.T.reshape(KC, P, 6 * C)).astype(bf),
        "qkv_wt": np.ascontiguousarray(qkv_w.T.reshape(KC, P, 3 * C)).astype(bf),
        "proj_wt": np.ascontiguousarray(proj_w.T.reshape(KC, P, C)).astype(bf),
        "fc1_wt": np.ascontiguousarray(fc1_w.T.reshape(KC, P, MLP)).astype(bf),
        "fc2_wt": np.ascontiguousarray(fc2_w.T.reshape(MLP // P, P, C)).astype(bf),
        "qkv_b_qk": np.ascontiguousarray(qkv_b[:2 * C].reshape(8, P).T).astype(f32),
        "fc1_b_c": np.ascontiguousarray(fc1_b.reshape(MLP // P, P).T).astype(f32),
        "vb_row": qkv_b[2 * C:].reshape(1, C).astype(f32),
    }
    # host-folded constants (weights-only algebra; inputs never touched):
    #   W = ln_w*(1+mod_sc) where mod_sc = dev_sc + ada_b_sc
    #     = dev_sc*A + D with A = ln_w, D = ln_w*(1+ada_b_sc); similarly B, G.
    # column layout: vec[c] at [c % 128, c // 128]
    def col(v):
        return np.ascontiguousarray(v.reshape(KC, P).T).astype(f32)

    lncols = []
    for br, (lnw, lnb, pb) in {1: (ln["ln1_w"], ln["ln1_b"], proj_b),
                               2: (ln["ln2_w"], ln["ln2_b"], fc2_b)}.items():
        o = (br - 1) * 3 * C
        sh_ab = ada_b[o:o + C]
        sc_ab = ada_b[o + C:o + 2 * C]
        g_ab = ada_b[o + 2 * C:o + 3 * C]
        lncols += [col(lnw), col(lnw * (1 + sc_ab)), col(lnb),
                   col(lnb * (1 + sc_ab) + sh_ab)]
        shared[f"gb{br}"] = g_ab.reshape(1, C).astype(f32)
        shared[f"pb{br}"] = pb.reshape(1, C).astype(f32)
    shared["lncols"] = np.ascontiguousarray(np.concatenate(lncols, axis=1))
    maps = []
    for b in range(B):
        m = dict(shared)
        m["x"] = np.ascontiguousarray(x[b].reshape(NT, P, C))
        m["c_col"] = np.ascontiguousarray(c[b].reshape(KC, P).T)
        maps.append(m)
    return maps


_CACHED_NC = None


def run(inputs, trace=False):
    global _CACHED_NC
    if _CACHED_NC is None:
        _CACHED_NC = build_program()
    maps = make_in_maps(inputs)
    res = run_bass_kernel_spmd(_CACHED_NC, maps, core_ids=list(range(B)),
                               trace=trace)
    out = np.stack([res.results[b]["out"].reshape(T, C) for b in range(B)])
    return out.astype(np.float32), res


def kernel(**inputs) -> np.ndarray:
    out, _ = run(inputs, trace=False)
    return out
